# revision 10
# baseline (speedup 1.0000x reference)
"""AutoCov1D Trainium2 kernel (8 NeuronCores, data-parallel over batch).

Math: for window n (stride 8, width 64), with X1 = X[:, :-64], X2 = X[:, 64:]:
  p1 = einsum('bnw,wdc', X1win, Wgt); p2 likewise with X2win
  out = mean_d(p1c * p2c) + bias   (p*c centered over d)

Exact simplifications:
  1. Centering over d is linear in the weight, so pre-center the weight:
     Wtil = (W - mean_d W) / sqrt(D); then no mean terms remain.
  2. X2 windows are X1 windows shifted by 8 window indices (64 = 8*stride),
     so ONE projection P[b,m,:] = sum_w X[b, 8m+w] * Wtil[w,:] over m=0..504
     serves both operands:  out[b,n,c] = sum_d P[b,n,d,c]*P[b,n+8,d,c] + bias.

V2 performance structure (per core, B_shard=4 processed as 2 row-PAIRS):
  - Polyphase X staging: xpoly[w, m] = X[8m + w] -> matmul rhs reads are
    contiguous (stride 1), avoiding the 2 cyc/row strided-read penalty.
  - Row-tiled projections: pair rhs lives on partitions 0-63 (row b0) and
    64-127 (row b1) with duplicated weights; the two K=64 matmuls go to PE
    row groups (0,0)/(64,0) and execute concurrently.
  - PSUM -> SBUF evacuation (fp32 -> bf16) split between ACT and DVE.
  - Shifted products P[n]*P[n+8] on DVE (bf16 2x) with a GPSIMD share.
  - Col-tiled selector matmuls (K=128 -> M=32 per 32-channel block,
    tile_position (0,32cb)) reduce the 4 in-tile latent dims and accumulate
    the 8 dq quads in PSUM fp32; groups of 4 issued back-to-back so the col
    groups overlap on the PE.
  - Emission order keeps PE streams back-to-back (HAM stays at K=8/8).
"""

import sys

import numpy as np

if "/opt/trn_rl_repo" not in sys.path:
    sys.path.insert(0, "/opt/trn_rl_repo")

_B, _T, _W, _D, _C = 32, 4096, 64, 32, 128
_NCORES = 8
_BSH = _B // _NCORES  # 4 rows per core -> 2 pairs
_NPAIR = _BSH // 2
_M = 505  # projection windows per batch row
_N = 497  # output windows per batch row
_S = 8  # stride

# engine-split knobs (unit = one (dq, cb) block of a pair)
_EVAC_DVE = {3}  # u % 4 in this set -> evac on DVE, else ACT
# GPSIMD products only on the FIRST unit of each dq group: its ~2us op has
# ~5 unit-times of slack before that group's selector matmuls need it
_PROD_GP = {0}  # u % 4 in this set -> products on GPSIMD, else DVE
# tail units (last 4 of each pair): spread the drain across engine FIFOs
_TAIL_EVAC = {28: "act", 29: "act", 30: "dve", 31: "act"}
_TAIL_PROD = {28: "gp", 29: "dve", 30: "dve", 31: "dve"}
_SEL_LAG = 2  # units between last proj of a dq group and its sel emission

_NC_CACHE = None


def _build_nc():
    import concourse.bass as bass
    import concourse.tile as tile
    from concourse import bacc, mybir
    from contextlib import ExitStack

    f32 = mybir.dt.float32
    bf16 = mybir.dt.bfloat16

    nc = bacc.Bacc(None, target_bir_lowering=False)
    # xsh[pair, 0:64, m] = X[b0, 8m+w]; xsh[pair, 64:128, m] = X[b1, 8m+w]
    x = nc.declare_dram_parameter("xsh", [_NPAIR, 128, _M], bf16, isOutput=False)
    # wt[w & w+64, dq, cb, dd*32+cc] = Wtil[w, 4*dq+dd, 32*cb+cc] (dup halves)
    wt = nc.declare_dram_parameter("wt", [128, 8, 4, _C], bf16, isOutput=False)
    sel = nc.declare_dram_parameter("sel", [_C, 32], bf16, isOutput=False)
    bias = nc.declare_dram_parameter("bias", [_C, 1], f32, isOutput=False)
    out = nc.declare_dram_parameter("out", [_BSH, _C, _N], f32, isOutput=True)

    with ExitStack() as ctx:
        tc = ctx.enter_context(tile.TileContext(nc))
        singles = ctx.enter_context(tc.tile_pool(name="singles", bufs=1))
        psp = ctx.enter_context(tc.tile_pool(name="psp", bufs=3, space="PSUM"))
        covp = ctx.enter_context(tc.tile_pool(name="covp", bufs=1, space="PSUM"))
        evacp = ctx.enter_context(tc.tile_pool(name="evacp", bufs=6))
        prodp = ctx.enter_context(tc.tile_pool(name="prodp", bufs=10))
        outp = ctx.enter_context(tc.tile_pool(name="outp", bufs=2))

        # first-needed first: xp0 + wt_g0 gate the first matmul
        xp_tiles = [
            singles.tile([128, _M], bf16, name=f"xp{p}", tag=f"xp{p}")
            for p in range(_NPAIR)
        ]
        wt_tiles = [
            singles.tile([128, 4, 4, _C], bf16, name=f"wtg{g}", tag=f"wtg{g}")
            for g in range(2)
        ]
        nc.sync.dma_start(out=xp_tiles[0], in_=x[0])
        nc.sync.dma_start(out=wt_tiles[0], in_=wt[:, 0:4, :, :])
        sel_sb = singles.tile([_C, 32], bf16)
        nc.sync.dma_start(out=sel_sb, in_=sel[:, :])
        bias_sb = singles.tile([_C, 1], f32)
        nc.sync.dma_start(out=bias_sb, in_=bias[:, :])
        nc.sync.dma_start(out=wt_tiles[1], in_=wt[:, 4:8, :, :])
        for p in range(1, _NPAIR):
            nc.sync.dma_start(out=xp_tiles[p], in_=x[p])

        for p in range(_NPAIR):
            xpair = xp_tiles[p]
            cov = covp.tile([_C, 2, 512], f32)
            pr_tiles = {}

            def emit_sel_group(dq):
                for b in range(2):
                    for cb in range(4):
                        nc.tensor.matmul(
                            cov[32 * cb : 32 * cb + 32, b, 0:_N],
                            lhsT=sel_sb[:, :],
                            rhs=pr_tiles[(dq, cb)][:, b, 0:_N],
                            start=(dq == 0),
                            stop=(dq == 7),
                            tile_position=(0, 32 * cb),
                        )
                for cb in range(4):
                    del pr_tiles[(dq, cb)]

            for u in range(32):
                dq, cb = divmod(u, 4)
                ps = psp.tile([128, 2, 512], f32)
                for j in range(2):
                    nc.tensor.matmul(
                        ps[:, j, 0:_M],
                        lhsT=wt_tiles[dq // 4][
                            64 * j : 64 * j + 64, dq % 4, cb, :
                        ],
                        rhs=xpair[64 * j : 64 * j + 64, :],
                        start=True,
                        stop=True,
                    )
                evac_eng = _TAIL_EVAC.get(
                    u, "dve" if (u % 4) in _EVAC_DVE else "act"
                )
                prod_eng = _TAIL_PROD.get(
                    u, "gp" if (u % 4) in _PROD_GP else "dve"
                )
                ev = evacp.tile([128, 2, _M], bf16)
                if evac_eng == "dve":
                    nc.vector.tensor_copy(ev[:, :, :], ps[:, :, 0:_M])
                else:
                    nc.scalar.copy(out=ev[:, :, :], in_=ps[:, :, 0:_M])
                pr = prodp.tile([128, 2, _N], bf16)
                if prod_eng == "gp":
                    nc.gpsimd.tensor_mul(
                        pr[:, :, :], ev[:, :, 0:_N], ev[:, :, _S:_M]
                    )
                else:
                    nc.vector.tensor_mul(
                        pr[:, :, :], ev[:, :, 0:_N], ev[:, :, _S:_M]
                    )
                pr_tiles[(dq, cb)] = pr
                # emit completed dq group's selector matmuls, lagged
                udone = u - _SEL_LAG
                if udone >= 3 and udone % 4 == 3:
                    emit_sel_group(udone // 4)
            for dq in range(8 - (_SEL_LAG + 3) // 4, 8):
                emit_sel_group(dq)

            # per-b bias+store so b0's DMA overlaps b1's bias-add
            ot = outp.tile([_C, 2, _N], f32)
            for b in range(2):
                nc.vector.tensor_scalar_add(
                    ot[:, b, :], cov[:, b, 0:_N], bias_sb[:, 0:1]
                )
                nc.sync.dma_start(out=out[2 * p + b], in_=ot[:, b, :])
    nc.finalize()
    return nc


def _prep_inputs(X, weight, bias):
    import ml_dtypes

    X = np.asarray(X, dtype=np.float32)
    weight = np.asarray(weight, dtype=np.float32)
    bias = np.asarray(bias, dtype=np.float32)

    wtil = (weight - weight.mean(axis=1, keepdims=True)) / np.sqrt(np.float32(_D))
    # regroup to [w, dq, cb, dd*32+cc], duplicate along w for row groups
    wsel = (
        wtil.reshape(_W, 8, 4, 4, 32)  # w, dq, dd, cb, cc
        .transpose(0, 1, 3, 2, 4)  # w, dq, cb, dd, cc
        .reshape(_W, 8, 4, _C)
    )
    wdup = np.concatenate([wsel, wsel], axis=0)  # [128, 8, 4, C]
    wdup = np.ascontiguousarray(wdup).astype(ml_dtypes.bfloat16)

    # polyphase: xpoly[b, w, m] = X[b, 8m + w] (zero-padded past T)
    Xp = np.zeros((_B, _S * _M + _W), dtype=np.float32)
    Xp[:, :_T] = X
    idx = np.arange(_M)[None, :] * _S + np.arange(_W)[:, None]  # [w, m]
    xpoly = Xp[:, idx].astype(ml_dtypes.bfloat16)  # [B, 64, M]

    selm = np.zeros((_C, 32), dtype=np.float32)
    for q in range(_C):
        selm[q, q % 32] = 1.0
    selm = selm.astype(ml_dtypes.bfloat16)

    bias2 = np.ascontiguousarray(bias.reshape(_C, 1))

    in_maps = []
    for k in range(_NCORES):
        rows = xpoly[k * _BSH : (k + 1) * _BSH]  # [4, 64, M]
        xsh = rows.reshape(_NPAIR, 128, _M)  # pair p: rows 2p (top), 2p+1 (bottom)
        in_maps.append(
            {
                "xsh": np.ascontiguousarray(xsh),
                "wt": wdup,
                "sel": selm,
                "bias": bias2,
            }
        )
    return in_maps


def get_nc():
    global _NC_CACHE
    if _NC_CACHE is None:
        _NC_CACHE = _build_nc()
    return _NC_CACHE


def run(X, weight, bias, trace=False, tmpdir=None):
    """Returns (full_output, BassKernelResults)."""
    from concourse.bass_utils import run_bass_kernel_spmd

    nc = get_nc()
    in_maps = _prep_inputs(X, weight, bias)
    res = run_bass_kernel_spmd(
        nc, in_maps, core_ids=list(range(_NCORES)), trace=trace, tmpdir=tmpdir
    )
    parts = [res.results[i]["out"].transpose(0, 2, 1) for i in range(_NCORES)]
    full = np.ascontiguousarray(np.concatenate(parts, axis=0), dtype=np.float32)
    return full, res


def kernel(X, weight, bias):
    full, _ = run(X, weight, bias)
    return full


# revision 14
# speedup vs baseline: 1.0308x; 1.0308x over previous
"""AutoCov1D Trainium2 kernel (8 NeuronCores, data-parallel over batch).

Math: for window n (stride 8, width 64), with X1 = X[:, :-64], X2 = X[:, 64:]:
  p1 = einsum('bnw,wdc', X1win, Wgt); p2 likewise with X2win
  out = mean_d(p1c * p2c) + bias   (p*c centered over d)

Exact simplifications:
  1. Centering over d is linear in the weight, so pre-center the weight:
     Wtil = (W - mean_d W) / sqrt(D); then no mean terms remain.
  2. X2 windows are X1 windows shifted by 8 window indices (64 = 8*stride),
     so ONE projection P[b,m,:] = sum_w X[b, 8m+w] * Wtil[w,:] over m=0..504
     serves both operands:  out[b,n,c] = sum_d P[b,n,d,c]*P[b,n+8,d,c] + bias.

V2 performance structure (per core, B_shard=4 processed as 2 row-PAIRS):
  - Polyphase X staging: xpoly[w, m] = X[8m + w] -> matmul rhs reads are
    contiguous (stride 1), avoiding the 2 cyc/row strided-read penalty.
  - Row-tiled projections: pair rhs lives on partitions 0-63 (row b0) and
    64-127 (row b1) with duplicated weights; the two K=64 matmuls go to PE
    row groups (0,0)/(64,0) and execute concurrently.
  - PSUM -> SBUF evacuation (fp32 -> bf16) split between ACT and DVE.
  - Shifted products P[n]*P[n+8] on DVE (bf16 2x) with a GPSIMD share.
  - Col-tiled selector matmuls (K=128 -> M=32 per 32-channel block,
    tile_position (0,32cb)) reduce the 4 in-tile latent dims and accumulate
    the 8 dq quads in PSUM fp32; groups of 4 issued back-to-back so the col
    groups overlap on the PE.
  - Emission order keeps PE streams back-to-back (HAM stays at K=8/8).
"""

import sys

import numpy as np

if "/opt/trn_rl_repo" not in sys.path:
    sys.path.insert(0, "/opt/trn_rl_repo")

_B, _T, _W, _D, _C = 32, 4096, 64, 32, 128
_NCORES = 8
_BSH = _B // _NCORES  # 4 rows per core -> 2 pairs
_NPAIR = _BSH // 2
_M = 505  # projection windows per batch row
_N = 497  # output windows per batch row
_S = 8  # stride

# engine-split knobs (unit = one (dq, cb) block of a pair)
# per-unit product engine: GPSIMD on the first unit of each dq group (its
# ~2us op then has ~5 unit-times of slack before that group's sel matmuls);
# DVE (bf16 2x) elsewhere. NOTE: both-operands-from-PSUM DVE ops are illegal
# on TRN2 (NCC_IBVF027), so every unit goes through an SBUF evacuation.
def _prod_eng(u):
    return "gp" if u % 4 == 0 else "dve"


# evacuation engine: mostly ACT (it has no other work); a slice on DVE to
# keep ACT off the pipeline critical path
def _evac_eng(u):
    return "dve" if (u % 8 == 3 or u % 16 == 7) else "act"
_SEL_LAG = 2  # units between last proj of a dq group and its sel emission

_NC_CACHE = None


def _build_nc():
    import concourse.bass as bass
    import concourse.tile as tile
    from concourse import bacc, mybir
    from contextlib import ExitStack

    f32 = mybir.dt.float32
    bf16 = mybir.dt.bfloat16

    nc = bacc.Bacc(None, target_bir_lowering=False)
    # xsh[pair, 0:64, m] = X[b0, 8m+w]; xsh[pair, 64:128, m] = X[b1, 8m+w]
    x = nc.declare_dram_parameter("xsh", [_NPAIR, 128, _M], bf16, isOutput=False)
    # wt[w & w+64, dq, cb, dd*32+cc] = Wtil[w, 4*dq+dd, 32*cb+cc] (dup halves)
    wt = nc.declare_dram_parameter("wt", [128, 8, 4, _C], bf16, isOutput=False)
    sel = nc.declare_dram_parameter("sel", [_C, 32], bf16, isOutput=False)
    bias = nc.declare_dram_parameter("bias", [_C, 1], f32, isOutput=False)
    out = nc.declare_dram_parameter("out", [_BSH, _C, _N], f32, isOutput=True)

    with ExitStack() as ctx:
        tc = ctx.enter_context(tile.TileContext(nc))
        singles = ctx.enter_context(tc.tile_pool(name="singles", bufs=1))
        psp = ctx.enter_context(tc.tile_pool(name="psp", bufs=3, space="PSUM"))
        covp = ctx.enter_context(tc.tile_pool(name="covp", bufs=1, space="PSUM"))
        evacp = ctx.enter_context(tc.tile_pool(name="evacp", bufs=6))
        prodp = ctx.enter_context(tc.tile_pool(name="prodp", bufs=10))
        outp = ctx.enter_context(tc.tile_pool(name="outp", bufs=2))

        # first-needed first: xp0 + wt_g0 gate the first matmul
        xp_tiles = [
            singles.tile([128, _M], bf16, name=f"xp{p}", tag=f"xp{p}")
            for p in range(_NPAIR)
        ]
        wt_tiles = [
            singles.tile([128, 4, 4, _C], bf16, name=f"wtg{g}", tag=f"wtg{g}")
            for g in range(2)
        ]
        nc.sync.dma_start(out=xp_tiles[0], in_=x[0])
        nc.sync.dma_start(out=wt_tiles[0], in_=wt[:, 0:4, :, :])
        sel_sb = singles.tile([_C, 32], bf16)
        nc.sync.dma_start(out=sel_sb, in_=sel[:, :])
        bias_sb = singles.tile([_C, 1], f32)
        nc.sync.dma_start(out=bias_sb, in_=bias[:, :])
        nc.sync.dma_start(out=wt_tiles[1], in_=wt[:, 4:8, :, :])
        for p in range(1, _NPAIR):
            nc.sync.dma_start(out=xp_tiles[p], in_=x[p])

        for p in range(_NPAIR):
            xpair = xp_tiles[p]
            cov = covp.tile([_C, 2, 512], f32)
            pr_tiles = {}

            def emit_sel_group(dq):
                for b in range(2):
                    for cb in range(4):
                        nc.tensor.matmul(
                            cov[32 * cb : 32 * cb + 32, b, 0:_N],
                            lhsT=sel_sb[:, :],
                            rhs=pr_tiles[(dq, cb)][:, b, 0:_N],
                            start=(dq == 0),
                            stop=(dq == 7),
                            tile_position=(0, 32 * cb),
                        )
                for cb in range(4):
                    del pr_tiles[(dq, cb)]

            for u in range(32):
                dq, cb = divmod(u, 4)
                ps = psp.tile([128, 2, 512], f32)
                for j in range(2):
                    nc.tensor.matmul(
                        ps[:, j, 0:_M],
                        lhsT=wt_tiles[dq // 4][
                            64 * j : 64 * j + 64, dq % 4, cb, :
                        ],
                        rhs=xpair[64 * j : 64 * j + 64, :],
                        start=True,
                        stop=True,
                    )
                pr = prodp.tile([128, 2, _N], bf16)
                ev = evacp.tile([128, 2, _M], bf16)
                if _evac_eng(u) == "dve":
                    nc.vector.tensor_copy(ev[:, :, :], ps[:, :, 0:_M])
                else:
                    nc.scalar.copy(out=ev[:, :, :], in_=ps[:, :, 0:_M])
                if _prod_eng(u) == "gp":
                    nc.gpsimd.tensor_mul(
                        pr[:, :, :], ev[:, :, 0:_N], ev[:, :, _S:_M]
                    )
                else:
                    nc.vector.tensor_mul(
                        pr[:, :, :], ev[:, :, 0:_N], ev[:, :, _S:_M]
                    )
                pr_tiles[(dq, cb)] = pr
                # emit completed dq group's selector matmuls, lagged
                udone = u - _SEL_LAG
                if udone >= 3 and udone % 4 == 3:
                    emit_sel_group(udone // 4)
            for dq in range(8 - (_SEL_LAG + 3) // 4, 8):
                emit_sel_group(dq)

            # per-b bias+store so b0's DMA overlaps b1's bias-add
            ot = outp.tile([_C, 2, _N], f32)
            for b in range(2):
                nc.vector.tensor_scalar_add(
                    ot[:, b, :], cov[:, b, 0:_N], bias_sb[:, 0:1]
                )
                nc.sync.dma_start(out=out[2 * p + b], in_=ot[:, b, :])
    nc.finalize()
    return nc


def _prep_inputs(X, weight, bias):
    import ml_dtypes

    X = np.asarray(X, dtype=np.float32)
    weight = np.asarray(weight, dtype=np.float32)
    bias = np.asarray(bias, dtype=np.float32)

    wtil = (weight - weight.mean(axis=1, keepdims=True)) / np.sqrt(np.float32(_D))
    # regroup to [w, dq, cb, dd*32+cc], duplicate along w for row groups
    wsel = (
        wtil.reshape(_W, 8, 4, 4, 32)  # w, dq, dd, cb, cc
        .transpose(0, 1, 3, 2, 4)  # w, dq, cb, dd, cc
        .reshape(_W, 8, 4, _C)
    )
    wdup = np.concatenate([wsel, wsel], axis=0)  # [128, 8, 4, C]
    wdup = np.ascontiguousarray(wdup).astype(ml_dtypes.bfloat16)

    # polyphase: xpoly[b, w, m] = X[b, 8m + w] (zero-padded past T)
    Xp = np.zeros((_B, _S * _M + _W), dtype=np.float32)
    Xp[:, :_T] = X
    idx = np.arange(_M)[None, :] * _S + np.arange(_W)[:, None]  # [w, m]
    xpoly = Xp[:, idx].astype(ml_dtypes.bfloat16)  # [B, 64, M]

    selm = np.zeros((_C, 32), dtype=np.float32)
    for q in range(_C):
        selm[q, q % 32] = 1.0
    selm = selm.astype(ml_dtypes.bfloat16)

    bias2 = np.ascontiguousarray(bias.reshape(_C, 1))

    in_maps = []
    for k in range(_NCORES):
        rows = xpoly[k * _BSH : (k + 1) * _BSH]  # [4, 64, M]
        xsh = rows.reshape(_NPAIR, 128, _M)  # pair p: rows 2p (top), 2p+1 (bottom)
        in_maps.append(
            {
                "xsh": np.ascontiguousarray(xsh),
                "wt": wdup,
                "sel": selm,
                "bias": bias2,
            }
        )
    return in_maps


def get_nc():
    global _NC_CACHE
    if _NC_CACHE is None:
        _NC_CACHE = _build_nc()
    return _NC_CACHE


def run(X, weight, bias, trace=False, tmpdir=None):
    """Returns (full_output, BassKernelResults)."""
    from concourse.bass_utils import run_bass_kernel_spmd

    nc = get_nc()
    in_maps = _prep_inputs(X, weight, bias)
    res = run_bass_kernel_spmd(
        nc, in_maps, core_ids=list(range(_NCORES)), trace=trace, tmpdir=tmpdir
    )
    parts = [res.results[i]["out"].transpose(0, 2, 1) for i in range(_NCORES)]
    full = np.ascontiguousarray(np.concatenate(parts, axis=0), dtype=np.float32)
    return full, res


def kernel(X, weight, bias):
    full, _ = run(X, weight, bias)
    return full
